# revision 3
# baseline (speedup 1.0000x reference)
"""Trainium2 Bass kernel for ContinualLoraMoeOneGateInjectedLinear.

Computation (see reference):
    route  = lora_route[task_id-1]            (or sum over tasks if task_id > 5)
    a      = x @ route                        [B,S,P]
    gate   = 2*mean(softmax(a, -1), S) - 1    [B,P]
    tid    = min(task_id, 5)
    delta  = sum_p gate[b,p] * (x @ down_p) @ up_p   (p < tid)
    y      = x @ linear_w.T + delta

Sharding: 8 cores = 4 batches x 2 output-halves.  Core k handles batch
k//2 and output columns [512*(k%2), 512*(k%2+1)).  Each core sees all
tokens of its batch, so the gate is computed locally - no collectives.

Device kernel (per core), all heavy matmuls in float32r:
  1. zaT[45, 4096] = [down|route].T @ x   (one pass over x)
  2. transpose routing logits to [4096, 5], softmax, token-sum -> gate[5]
  3. y_base[4096, 512] = x @ Wt_half      (streams concurrently)
  4. y = y_base + zT.T @ (gate-scaled up) (added during PSUM drain)
"""

import sys

if "/opt/trn_rl_repo" not in sys.path:
    sys.path.insert(0, "/opt/trn_rl_repo")

from contextlib import ExitStack

import numpy as np

import concourse.bass as bass
import concourse.mybir as mybir
import concourse.tile as tile
from concourse import bacc
from concourse.bass_utils import run_bass_kernel_spmd

F32 = mybir.dt.float32
F32R = mybir.dt.float32r

NUM_TASKS = 5
B, S, IN, OUT, P, R = 4, 4096, 1024, 1024, 5, 8
RT = P * R  # 40 total low-rank dims
ZA = 72  # fused [down|route] matmul rows: 0:40 down, 64:69 route (32-aligned)
RB = 64  # partition base of the route rows
OH = OUT // 2  # 512, per-core output half
NI = IN // 128  # 8 i-tiles
NC = S // 512  # 8 token chunks of 512
NG = S // 128  # 32 token tiles of 128


def build_kernel():
    """Build the per-core Bacc module (identical program on all 8 cores)."""
    nc = bacc.Bacc("TRN2", target_bir_lowering=False, debug=False, num_devices=8)

    xt_d = nc.dram_tensor("xt", [IN, S], F32R, kind="ExternalInput").ap()
    wt_d = nc.dram_tensor("wt", [IN, OH], F32R, kind="ExternalInput").ap()
    rd_d = nc.dram_tensor("rd", [IN, ZA], F32R, kind="ExternalInput").ap()
    up_d = nc.dram_tensor("up", [RT, OH], F32, kind="ExternalInput").ap()
    eye_d = nc.dram_tensor("eye5", [P, P], F32, kind="ExternalInput").ap()
    ones_d = nc.dram_tensor("ones", [128, 1], F32, kind="ExternalInput").ap()
    e40_d = nc.dram_tensor("e40", [P, RT], F32, kind="ExternalInput").ap()
    y_d = nc.dram_tensor("y", [S, OH], F32, kind="ExternalOutput").ap()

    with tile.TileContext(nc) as tc, ExitStack() as ctx:
        consts = ctx.enter_context(tc.tile_pool(name="consts", bufs=1))
        rdp = ctx.enter_context(tc.tile_pool(name="rdp", bufs=NI))
        wp = ctx.enter_context(tc.tile_pool(name="wp", bufs=NI))
        xp = ctx.enter_context(tc.tile_pool(name="xp", bufs=24))
        zp = ctx.enter_context(tc.tile_pool(name="zp", bufs=1))
        sfx = ctx.enter_context(tc.tile_pool(name="sfx", bufs=1))
        yb = ctx.enter_context(tc.tile_pool(name="yb", bufs=NG))
        za_ps = ctx.enter_context(tc.tile_pool(name="za_ps", bufs=2, space="PSUM"))
        y_ps = ctx.enter_context(tc.tile_pool(name="y_ps", bufs=4, space="PSUM"))
        tr_ps = ctx.enter_context(tc.tile_pool(name="tr_ps", bufs=1, space="PSUM"))
        sm_ps = ctx.enter_context(tc.tile_pool(name="sm_ps", bufs=1, space="PSUM"))

        # constants / small weights
        eye5 = consts.tile([P, P], F32)
        ones = consts.tile([128, 1], F32)
        e40 = consts.tile([P, RT], F32)
        up_sb = consts.tile([RT, OH], F32)
        for t, d in [(eye5, eye_d), (ones, ones_d), (e40, e40_d), (up_sb, up_d)]:
            nc.sync.dma_start(t[:], d)
        rd_t = []
        for i in range(NI):
            t = rdp.tile([128, ZA], F32R, tag="rd_t")
            nc.sync.dma_start(t[:], rd_d[128 * i : 128 * (i + 1), :])
            rd_t.append(t)
        w_t = []
        for i in range(NI):
            t = wp.tile([128, OH], F32R, tag="w_t")
            nc.sync.dma_start(t[:], wt_d[128 * i : 128 * (i + 1), :])
            w_t.append(t)

        # streamed x^T tiles, chunk-major
        xt_t = {}
        for c in range(NC):
            for i in range(NI):
                t = xp.tile([128, 512], F32R, tag="xt_t")
                nc.sync.dma_start(
                    t[:], xt_d[128 * i : 128 * (i + 1), 512 * c : 512 * (c + 1)]
                )
                xt_t[(i, c)] = t

        # fused [down|route] matmul + per-chunk drain + routing transposes
        zt_sb = zp.tile([RT, S], F32R)  # z^T, feeds the delta matmul
        at_sb = zp.tile([P, S], F32)  # routing logits a^T
        trp = tr_ps.tile([128, P * NG], F32)  # a, token-major, [128, 160]
        for c in range(NC):
            za = za_ps.tile([ZA, 512], F32, tag="za")
            for i in range(NI):
                nc.tensor.matmul(
                    za[:], rd_t[i][:], xt_t[(i, c)][:], start=(i == 0), stop=(i == NI - 1)
                )
            nc.any.tensor_copy(zt_sb[:, 512 * c : 512 * (c + 1)], za[0:RT, :])
            nc.any.tensor_copy(at_sb[:, 512 * c : 512 * (c + 1)], za[RB : RB + P, :])
            for q in range(4):
                g = 4 * c + q
                nc.tensor.transpose(
                    trp[:, P * g : P * (g + 1)],
                    at_sb[:, 128 * g : 128 * (g + 1)],
                    eye5[:],
                )

        # softmax over experts (max-free: |a| < ~4) and token partial sums
        e_sb = sfx.tile([128, P * NG], F32)
        nc.scalar.activation(e_sb[:], trp[:], mybir.ActivationFunctionType.Exp)
        den = sfx.tile([128, NG], F32)
        nc.vector.tensor_reduce(
            den[:],
            e_sb[:].rearrange("p (g f) -> p g f", f=P),
            axis=mybir.AxisListType.X,
            op=mybir.AluOpType.add,
        )
        invd = sfx.tile([128, NG], F32)
        nc.vector.reciprocal(invd[:], den[:])
        om = sfx.tile([128, P * NG], F32)
        nc.vector.tensor_tensor(
            om[:].rearrange("p (g f) -> p g f", f=P),
            e_sb[:].rearrange("p (g f) -> p g f", f=P),
            invd[:].unsqueeze(2).to_broadcast((128, NG, P)),
            mybir.AluOpType.mult,
        )
        pp = sm_ps.tile([P, 1], F32, tag="sm")
        for g in range(NG):
            nc.tensor.matmul(
                pp[:], om[:, P * g : P * (g + 1)], ones[:], start=(g == 0), stop=(g == NG - 1)
            )
        g5 = sfx.tile([P, 1], F32)
        # gate = 2*mean(omega over S) - 1
        nc.scalar.activation(
            g5[:], pp[:], mybir.ActivationFunctionType.Copy, bias=-1.0, scale=2.0 / S
        )
        ep = sm_ps.tile([RT, 1], F32, tag="sm")
        nc.tensor.matmul(ep[:], e40[:], g5[:], start=True, stop=True)
        g40 = sfx.tile([RT, 1], F32)
        nc.any.tensor_copy(g40[:], ep[:])
        upeff = sfx.tile([RT, OH], F32R)
        nc.vector.tensor_scalar_mul(upeff[:], up_sb[:], g40[:])

        # main y = x @ W matmuls, drained to SBUF as they complete
        y_sb = []
        for c in range(NC):
            for q in range(4):
                g = 4 * c + q
                ypt = y_ps.tile([128, OH], F32, tag="ypt")
                for i in range(NI):
                    nc.tensor.matmul(
                        ypt[:],
                        xt_t[(i, c)][:, 128 * q : 128 * (q + 1)],
                        w_t[i][:],
                        start=(i == 0),
                        stop=(i == NI - 1),
                    )
                yt = yb.tile([128, OH], F32, tag="y_sb")
                nc.any.tensor_copy(yt[:], ypt[:])
                y_sb.append(yt)

        # delta = z^T.T @ (gate * up), fused into the writeback
        for g in range(NG):
            dpt = y_ps.tile([128, OH], F32, tag="ypt")
            nc.tensor.matmul(
                dpt[:], zt_sb[:, 128 * g : 128 * (g + 1)], upeff[:], start=True, stop=True
            )
            nc.vector.tensor_add(y_sb[g][:], y_sb[g][:], dpt[:])
            nc.sync.dma_start(y_d[128 * g : 128 * (g + 1), :], y_sb[g][:])

    nc.compile()
    return nc


def _host_prep(inputs):
    """Shard/transform full inputs into the 8 per-core input maps."""
    x = np.asarray(inputs["input"], dtype=np.float32).reshape(B, S, IN)
    linear_w = np.asarray(inputs["linear_w"], dtype=np.float32)
    lora_down = np.asarray(inputs["lora_down"], dtype=np.float32)
    lora_up = np.asarray(inputs["lora_up"], dtype=np.float32)
    lora_route = np.asarray(inputs["lora_route"], dtype=np.float32)
    task_id = int(np.asarray(inputs["task_id"]))

    if task_id <= NUM_TASKS:
        route = lora_route[task_id - 1]  # python negative-index semantics
    else:
        route = lora_route.sum(axis=0)
    tid = min(task_id, NUM_TASKS)

    up_cat = np.zeros((RT, OUT), dtype=np.float32)
    rd = np.zeros((IN, ZA), dtype=np.float32)  # [down | pad | route]
    for p in range(tid):
        rd[:, p * R : (p + 1) * R] = lora_down[p]
        up_cat[p * R : (p + 1) * R, :] = lora_up[p]
    rd[:, RB : RB + P] = route
    wt = np.ascontiguousarray(linear_w.T)  # [IN, OUT]
    eye5 = np.eye(P, dtype=np.float32)
    ones = np.ones((128, 1), dtype=np.float32)
    e40 = np.zeros((P, RT), dtype=np.float32)
    for p in range(P):
        e40[p, p * R : (p + 1) * R] = 1.0

    xts = [np.ascontiguousarray(x[b].T) for b in range(B)]
    wts = [np.ascontiguousarray(wt[:, h * OH : (h + 1) * OH]) for h in range(2)]
    ups = [np.ascontiguousarray(up_cat[:, h * OH : (h + 1) * OH]) for h in range(2)]

    in_maps = []
    for k in range(8):
        b, h = k // 2, k % 2
        in_maps.append(
            {
                "xt": xts[b],
                "wt": wts[h],
                "rd": rd,
                "up": ups[h],
                "eye5": eye5,
                "ones": ones,
                "e40": e40,
            }
        )
    return in_maps


def _assemble(results):
    out = np.empty((B, S, OUT), dtype=np.float32)
    for k in range(8):
        b, h = k // 2, k % 2
        out[b, :, h * OH : (h + 1) * OH] = results[k]["y"]
    return out


def kernel(**inputs) -> np.ndarray:
    nc = build_kernel()
    in_maps = _host_prep(inputs)
    res = run_bass_kernel_spmd(nc, in_maps, core_ids=list(range(8)))
    return _assemble(res.results)


if __name__ == "__main__":
    rng = np.random.default_rng(0)
    demo = {
        "input": rng.standard_normal((B, S, IN), dtype=np.float32),
        "linear_w": (rng.standard_normal((OUT, IN)) * 0.02).astype(np.float32),
        "lora_down": (rng.standard_normal((P, IN, R)) * 0.02).astype(np.float32),
        "lora_up": (rng.standard_normal((P, R, OUT)) * 0.02).astype(np.float32),
        "lora_route": (rng.standard_normal((P, IN, P)) * 0.02).astype(np.float32),
        "task_id": 5,
    }
    y = kernel(**demo)
    print("ok", y.shape, y.dtype)
